# revision 30
# baseline (speedup 1.0000x reference)
"""Trainium2 Bass kernel for nn_DigitByDigitValueHead.

Model (per token):
  ctx_in = concat(hidden, op_emb, pm_emb)            # [1536]
  context = gelu(LN(ctx_in @ ctx_w + ctx_b))         # [1024]
  sign    = gelu(LN(context @ sign_w1 + b)) @ sign_w2 + b2     # [3]
  digit_p = gelu(LN(context @ dig_w1[p] (+pos term) + b)) @ dig_w2[p] + b2
  aux     = gelu(context @ aux_w1 + b) @ aux_w2 + b2            # [1]

Sharding: core c handles batch row b=c (2048 tokens). operation_type is
per-row, so its embedding contribution folds into a per-core bias vector
(host precompute). pt_embed folds into PM = pt_embed @ ctx_w[1280:1536]
([10,1024]) gathered on-device via a one-hot matmul accumulated into the
same PSUM group as hidden @ ctx_w[:1024].

On-chip layout is feature-major ([feat partitions, tokens free]); hidden is
transposed host-side. All matmuls run fp32r (1 cycle/row at N=512).
LayerNorm reduces over the feature (partition) axis: column sums of x and
x^2 via ones-matmuls into [1,N] PSUM rows; the [1,N] rows are repacked to
[128,N/128] via a DRAM round-trip so the Newton-rsqrt finalize runs
partition-parallel on DVE; mu and rsqrt(v+eps) are then broadcast across
partitions with scaled-ones [1,128] matmuls into PSUM, ACT-evicted, and
applied by DVE; gelu (+LN affine) runs on ACT (all ACT funcs used -
Identity/Copy-like evicts, Square, Gelu - live in one table set).
"""

import numpy as np
from contextlib import ExitStack

import concourse.bacc as bacc
import concourse.tile as tile
from concourse import mybir
from concourse.bass_utils import run_bass_kernel_spmd

F32 = mybir.dt.float32
F32R = mybir.dt.float32r
I32 = mybir.dt.int32
AF = mybir.ActivationFunctionType
ALU = mybir.AluOpType

B, T, D = 8, 2048, 1024
DQ, DH, P = 256, 512, 6
NOPS, NPT = 9, 10
N_CORES = 8
EPS = 1e-5
MAGIC = 0x5F3759DF
NT = 512                  # tokens per tile (fp32 matmul free-dim max)
NL = 8                    # head layers: 0=sign, 1..6=digit p, 7=aux
AUX_L = 7
USE_GELU = True           # CoreSim has no Gelu; tests may flip to Identity

# bcast_rows input: [4, 128] with rows (1/1024, 1/512, sqrt(1024), sqrt(512))
BC_INVD_CTX, BC_INVD_HEAD, BC_SQD_CTX, BC_SQD_HEAD = range(4)


def _ln_finalize(nc, sb, magic, mu_pk, q_pk, dim, r_out):
    """Packed mean MU and column sumsq Q [128,J] ->
    r_out = rsqrt(Q - D*MU^2 + eps*D) = rsqrt(D*(var+eps)); 3 Newton
    iterations from the int bit-trick seed. r_out is F32R (PE-consumed)."""
    J = mu_pk.shape[1]
    dd = float(dim)
    tmp = sb.tile([128, J], F32, tag="fin_tmp")
    w = sb.tile([128, J], F32, tag="fin_w")
    nc.vector.tensor_mul(tmp, mu_pk, mu_pk)
    nc.vector.scalar_tensor_tensor(w, tmp, -dd, q_pk,
                                   op0=ALU.mult, op1=ALU.add)
    nc.vector.tensor_scalar_add(w, w, EPS * dd)
    ti = sb.tile([128, J], I32, tag="fin_ti")
    nc.vector.tensor_scalar(ti, w.bitcast(I32), 1, None,
                            op0=ALU.logical_shift_right)
    nc.vector.scalar_tensor_tensor(ti, ti, -1, magic, op0=ALU.mult, op1=ALU.add)
    r = ti.bitcast(F32)
    for it in range(3):
        nc.vector.tensor_mul(tmp, r, r)
        nc.vector.scalar_tensor_tensor(tmp, w, -0.5, tmp,
                                       op0=ALU.mult, op1=ALU.mult)
        nc.vector.tensor_scalar_add(tmp, tmp, 1.5)
        nc.vector.tensor_mul(r_out if it == 2 else r, r, tmp)


def build_program(t_core=T, n_layers=NL, debug=False):
    """Build the per-core Bacc program. t_core tokens, feature-major I/O."""
    nc = bacc.Bacc("TRN2", target_bir_lowering=False, debug=debug,
                   num_devices=N_CORES)
    ntt = t_core // NT
    gelu_fn = AF.Gelu if USE_GELU else AF.Identity

    # ---- I/O ----
    hiddenT = nc.dram_tensor("hiddenT", [D, t_core], F32R, kind="ExternalInput").ap()
    onehot = nc.dram_tensor("onehot", [NPT, t_core], F32R, kind="ExternalInput").ap()
    ctx_bias = nc.dram_tensor("ctx_bias", [D], F32, kind="ExternalInput").ap()
    ctx_g = nc.dram_tensor("ctx_g", [D], F32, kind="ExternalInput").ap()
    ctx_beta = nc.dram_tensor("ctx_beta", [D], F32, kind="ExternalInput").ap()
    wh = nc.dram_tensor("wh", [D, D], F32R, kind="ExternalInput").ap()
    pm = nc.dram_tensor("pm", [NPT, D], F32R, kind="ExternalInput").ap()
    lw_all = nc.dram_tensor("lw_all", [n_layers, D, DH], F32R, kind="ExternalInput").ap()
    lb_all = nc.dram_tensor("lb_all", [n_layers, DH], F32, kind="ExternalInput").ap()
    lg_all = nc.dram_tensor("lg_all", [n_layers, DH], F32, kind="ExternalInput").ap()
    lbeta_all = nc.dram_tensor("lbeta_all", [n_layers, DH], F32, kind="ExternalInput").ap()
    # w2 columns per layer: sign 3, aux 1, digit 10 -> padded to 16 cols
    w2_all = nc.dram_tensor("w2_all", [n_layers, DH, 16], F32R, kind="ExternalInput").ap()
    b2_all = nc.dram_tensor("b2_all", [n_layers, 16], F32, kind="ExternalInput").ap()
    ones_col = nc.dram_tensor("ones_col", [128, 1], F32R, kind="ExternalInput").ap()
    invd_mats = nc.dram_tensor("invd_mats", [2, 128, 128], F32R, kind="ExternalInput").ap()
    bcast_rows = nc.dram_tensor("bcast_rows", [4, 128], F32R, kind="ExternalInput").ap()

    out_sign = nc.dram_tensor("out_sign", [3, t_core], F32, kind="ExternalOutput").ap()
    out_aux = nc.dram_tensor("out_aux", [1, t_core], F32, kind="ExternalOutput").ap()
    out_dig = nc.dram_tensor("out_dig", [P, 10, t_core], F32, kind="ExternalOutput").ap()
    outs = [out_sign] + [out_dig[p] for p in range(P)] + [out_aux]

    KC = D // 128   # 8 contraction chunks of context/hidden
    MC = D // 128   # 8 ctx output chunks
    HC = DH // 128  # 4 head chunks
    J = NT // 128   # 4

    with tile.TileContext(nc) as tc, ExitStack() as ctx:
        const = ctx.enter_context(tc.tile_pool(name="const", bufs=1))
        sbs = ctx.enter_context(tc.tile_pool(name="sbs", bufs=1))       # stat rows
        fin = ctx.enter_context(tc.tile_pool(name="fin", bufs=2))       # finalize
        dram = ctx.enter_context(tc.tile_pool(name="dram", bufs=4, space="DRAM"))
        ps_mm = ctx.enter_context(tc.tile_pool(name="ps_mm", bufs=3, space="PSUM"))
        ps_st = ctx.enter_context(tc.tile_pool(name="ps_st", bufs=1, space="PSUM"))
        ps_bcm = ctx.enter_context(tc.tile_pool(name="ps_bcm", bufs=2, space="PSUM"))
        ps_bcr = ctx.enter_context(tc.tile_pool(name="ps_bcr", bufs=1, space="PSUM"))
        ps_w2 = ctx.enter_context(tc.tile_pool(name="ps_w2", bufs=1, space="PSUM"))
        ctxp = ctx.enter_context(tc.tile_pool(name="ctxp", bufs=1))
        wtp = ctx.enter_context(tc.tile_pool(name="wtp", bufs=3))       # wh halves + lw stream
        htp = ctx.enter_context(tc.tile_pool(name="htp", bufs=2))       # hiddenT chunks
        xhp = ctx.enter_context(tc.tile_pool(name="xhp", bufs=2))       # pre-LN x / h tiles
        x2p = ctx.enter_context(tc.tile_pool(name="x2p", bufs=2))
        ohp = ctx.enter_context(tc.tile_pool(name="ohp", bufs=2))
        outp = ctx.enter_context(tc.tile_pool(name="outp", bufs=3))

        hid_r = hiddenT.rearrange("(kc kp) t -> kp kc t", kp=128)

        def load_hT(tt):
            tsl = slice(tt * NT, (tt + 1) * NT)
            tiles = []
            for k in range(KC):
                t = htp.tile([128, NT], F32R, tag=f"hT{k}")
                nc.sync.dma_start(out=t, in_=hid_r[:, k, tsl])
                tiles.append(t)
            return tiles

        # critical-path loads first: hT0/wh chunk pairs, interleaved so they
        # spread across DMA queues
        wh_r = wh.rearrange("(kc kp) m -> kp kc m", kp=128)
        wh_ab = []
        for i in range(2):
            wh_half = wtp.tile([128, KC // 2, D], F32R, tag="wt")
            wh_ab.append(wh_half)
        hT0 = []
        H2 = NT // 2
        for k in range(KC):
            t = htp.tile([128, NT], F32R, tag=f"hT{k}")
            nc.sync.dma_start(out=t[:, 0:H2], in_=hid_r[:, k, 0:H2])
            nc.sync.dma_start(out=t[:, H2:NT], in_=hid_r[:, k, H2:NT])
            hT0.append(t)
            nc.sync.dma_start(out=wh_ab[k // 4][:, k % 4, 0:D // 2],
                              in_=wh_r[:, k, 0:D // 2])
            nc.sync.dma_start(out=wh_ab[k // 4][:, k % 4, D // 2:D],
                              in_=wh_r[:, k, D // 2:D])

        magic = const.tile([128, J], I32)
        nc.vector.memset(magic, MAGIC)
        ones_sb = const.tile([128, 1], F32R)
        nc.sync.dma_start(out=ones_sb, in_=ones_col)
        bcr_sb = []
        for i in range(4):
            t = const.tile([1, 128], F32R, tag=f"bcr{i}")
            nc.sync.dma_start(out=t, in_=bcast_rows[i:i + 1, :])
            bcr_sb.append(t)
        cb_sb = const.tile([128, MC], F32)
        nc.sync.dma_start(out=cb_sb, in_=ctx_bias.rearrange("(c p) -> p c", p=128))
        invd_sb = []
        for i in range(2):
            t = const.tile([128, 128], F32R, tag=f"invd{i}")
            nc.sync.dma_start(out=t, in_=invd_mats[i])
            invd_sb.append(t)
        pm_sb = const.tile([NPT, D], F32R)
        nc.sync.dma_start(out=pm_sb, in_=pm)

        def load_oh(tt):
            tsl = slice(tt * NT, (tt + 1) * NT)
            t = ohp.tile([NPT, NT], F32R, tag="oh")
            nc.sync.dma_start(out=t, in_=onehot[:, tsl])
            return t

        oh0 = load_oh(0)

        # remaining (non-critical) constants
        cg_sb = const.tile([128, MC], F32)
        nc.sync.dma_start(out=cg_sb, in_=ctx_g.rearrange("(c p) -> p c", p=128))
        cbe_sb = const.tile([128, MC], F32)
        nc.sync.dma_start(out=cbe_sb, in_=ctx_beta.rearrange("(c p) -> p c", p=128))
        lb_sb = const.tile([128, n_layers, HC], F32)
        nc.sync.dma_start(out=lb_sb, in_=lb_all.rearrange("l (c p) -> p l c", p=128))
        lg_sb = const.tile([128, n_layers, HC], F32)
        nc.sync.dma_start(out=lg_sb, in_=lg_all.rearrange("l (c p) -> p l c", p=128))
        lbe_sb = const.tile([128, n_layers, HC], F32)
        nc.sync.dma_start(out=lbe_sb, in_=lbeta_all.rearrange("l (c p) -> p l c", p=128))
        w2_sb = const.tile([128, n_layers, HC, 16], F32R)
        nc.sync.dma_start(out=w2_sb, in_=w2_all.rearrange("l (c p) o -> p l c o", p=128))
        b2_sb = const.tile([16, n_layers], F32)
        nc.sync.dma_start(out=b2_sb, in_=b2_all.rearrange("l o -> o l"))

        # context, resident (written as F32R by ACT gelu); one tile per
        # token-tile so head-layer reads only depend on that tile's writers
        context_t = []
        for i in range(ntt):
            ct = ctxp.tile([128, KC, NT], F32R, tag=f"ctx{i}")
            context_t.append(ct)

        def ln_chain(mu_row, q_row, dim, bc_sqd):
            """mu/sumsq rows [1,NT] -> rps [128,NT] PSUM broadcast of
            rsqrt(var+eps)."""
            scr_s = dram.tile([NT], F32, tag="scr_s")
            nc.sync.dma_start(out=scr_s.unsqueeze(0), in_=mu_row)
            scr_q = dram.tile([NT], F32, tag="scr_q")
            nc.sync.dma_start(out=scr_q.unsqueeze(0), in_=q_row)
            s_pk = fin.tile([128, J], F32, tag="s_pk")
            nc.sync.dma_start(out=s_pk, in_=scr_s.rearrange("(p j) -> p j", p=128))
            q_pk = fin.tile([128, J], F32, tag="q_pk")
            nc.sync.dma_start(out=q_pk, in_=scr_q.rearrange("(p j) -> p j", p=128))
            r_pk = fin.tile([128, J], F32R, tag="r_pk")
            _ln_finalize(nc, fin, magic, s_pk, q_pk, dim, r_pk)
            scr_r = dram.tile([NT], F32R, tag="scr_r")
            nc.sync.dma_start(out=scr_r.rearrange("(p j) -> p j", p=128), in_=r_pk)
            r_row = sbs.tile([1, NT], F32R, tag="r_row")
            nc.sync.dma_start(out=r_row, in_=scr_r.unsqueeze(0))
            rps = ps_bcr.tile([128, NT], F32, tag="rps")
            nc.tensor.matmul(rps, bcr_sb[bc_sqd], r_row,
                             start=True, stop=True)
            return rps

        # ==================== Phase 1: ctx layer ====================
        hts = {0: hT0}
        ohs = {0: oh0}
        for tt in range(ntt):
            tsl = slice(tt * NT, (tt + 1) * NT)
            hT = hts.pop(tt)
            oh_t = ohs.pop(tt)
            if tt + 1 < ntt:
                hts[tt + 1] = load_hT(tt + 1)
                ohs[tt + 1] = load_oh(tt + 1)

            mups = ps_bcm.tile([128, NT], F32, tag="mups")
            qps = ps_st.tile([1, NT], F32, tag="qps")
            xc = []
            x2c = []
            for m in range(MC):
                psum = ps_mm.tile([128, NT], F32, tag="mm")
                msl = slice(m * 128, (m + 1) * 128)
                for k in range(KC):
                    nc.tensor.matmul(psum, wh_ab[k // 4][:, k % 4, msl], hT[k],
                                     start=(k == 0), stop=False)
                nc.tensor.matmul(psum, pm_sb[:, msl], oh_t,
                                 start=False, stop=True)
                x = xhp.tile([128, NT], F32R, tag=f"xh{m}")
                nc.scalar.activation(x, psum, AF.Identity,
                                     bias=cb_sb[:, m:m + 1], scale=1.0)
                x2 = x2p.tile([128, NT], F32R, tag="x2")
                nc.scalar.activation(x2, psum, AF.Square,
                                     bias=cb_sb[:, m:m + 1], scale=1.0)
                nc.tensor.matmul(qps, ones_sb, x2,
                                 start=(m == 0), stop=(m == MC - 1))
                xc.append(x)

            for m in range(MC):
                nc.tensor.matmul(mups, invd_sb[0], xc[m],
                                 start=(m == 0), stop=(m == MC - 1))
            mu_row = sbs.tile([1, NT], F32, tag="mu_row")
            q_row = sbs.tile([1, NT], F32, tag="q_row")
            nc.vector.tensor_copy(mu_row, mups[0:1, :])
            nc.vector.tensor_copy(q_row, qps)
            r_sb = ln_chain(mu_row, q_row, D, BC_SQD_CTX)
            mu_sb = mups
            for m in range(MC):
                x = xc[m]
                nc.vector.tensor_sub(x, x.bitcast(F32), mu_sb)
                nc.vector.tensor_mul(x, x.bitcast(F32), r_sb)
                nc.scalar.activation(context_t[tt][:, m, :], x.bitcast(F32), gelu_fn,
                                     bias=cbe_sb[:, m:m + 1],
                                     scale=cg_sb[:, m:m + 1])

        # ==================== Phase 2: head layers ====================
        for L in range(n_layers):
            lw = wtp.tile([128, KC, DH], F32R, tag="wt")
            nc.sync.dma_start(
                out=lw, in_=lw_all[L].rearrange("(kc kp) m -> kp kc m", kp=128))
            n_out = 3 if L == 0 else (1 if L == AUX_L else 10)
            for tt in range(ntt):
                tsl = slice(tt * NT, (tt + 1) * NT)
                hs = []
                x2s = []
                if L != AUX_L:
                    mups = ps_bcm.tile([128, NT], F32, tag="mups")
                    qps = ps_st.tile([1, NT], F32, tag="qps")
                for m in range(HC):
                    psum = ps_mm.tile([128, NT], F32, tag="mm")
                    msl = slice(m * 128, (m + 1) * 128)
                    for k in range(KC):
                        nc.tensor.matmul(psum, lw[:, k, msl],
                                         context_t[tt][:, k, :],
                                         start=(k == 0), stop=(k == KC - 1))
                    h = xhp.tile([128, NT], F32R, tag=f"xh{m + 4 * (tt % 2)}")
                    if L == AUX_L:
                        nc.scalar.activation(h, psum, gelu_fn,
                                             bias=lb_sb[:, L, m:m + 1], scale=1.0)
                    else:
                        nc.scalar.activation(h, psum, AF.Identity,
                                             bias=lb_sb[:, L, m:m + 1], scale=1.0)
                        x2 = x2p.tile([128, NT], F32R, tag="x2")
                        nc.scalar.activation(x2, psum, AF.Square,
                                             bias=lb_sb[:, L, m:m + 1], scale=1.0)
                        nc.tensor.matmul(qps, ones_sb, x2,
                                         start=(m == 0), stop=(m == HC - 1))
                    hs.append(h)

                if L != AUX_L:
                    for m in range(HC):
                        nc.tensor.matmul(mups, invd_sb[1], hs[m],
                                         start=(m == 0), stop=(m == HC - 1))
                    mu_row = sbs.tile([1, NT], F32, tag="mu_row")
                    q_row = sbs.tile([1, NT], F32, tag="q_row")
                    nc.vector.tensor_copy(mu_row, mups[0:1, :])
                    nc.vector.tensor_copy(q_row, qps)
                    r_sb = ln_chain(mu_row, q_row, DH, BC_SQD_HEAD)
                    mu_sb = mups
                    for m in range(HC):
                        h = hs[m]
                        nc.vector.tensor_sub(h, h.bitcast(F32), mu_sb)
                        nc.vector.tensor_mul(h, h.bitcast(F32), r_sb)
                        nc.scalar.activation(h, h.bitcast(F32), gelu_fn,
                                             bias=lbe_sb[:, L, m:m + 1],
                                             scale=lg_sb[:, L, m:m + 1])

                psum2 = ps_w2.tile([n_out, NT], F32, tag="w2")
                for k in range(HC):
                    nc.tensor.matmul(psum2, w2_sb[:, L, k, :n_out], hs[k],
                                     start=(k == 0), stop=(k == HC - 1))
                ostage = outp.tile([n_out, NT], F32, tag="ost")
                nc.scalar.activation(ostage, psum2, AF.Identity,
                                     bias=b2_sb[:n_out, L:L + 1], scale=1.0)
                nc.sync.dma_start(out=outs[L][:, tsl], in_=ostage)

    nc.compile()
    return nc


# ----------------------------------------------------------------------------
# Host side
# ----------------------------------------------------------------------------

def _preprocess(inputs):
    f32 = np.float32
    hidden = np.asarray(inputs["hidden"], f32)
    op_t = np.asarray(inputs["operation_type"])
    pt = np.asarray(inputs["param_type"])
    ctx_w = np.asarray(inputs["ctx_w"], f32)
    dig_w1 = np.asarray(inputs["dig_w1"], f32)

    shared = {
        "wh": np.ascontiguousarray(ctx_w[:D]),
        "pm": np.ascontiguousarray(np.asarray(inputs["pt_embed"], f32) @ ctx_w[D + DQ:]),
        "ctx_g": np.asarray(inputs["ctx_g"], f32),
        "ctx_beta": np.asarray(inputs["ctx_beta"], f32),
        "ones_col": np.ones((128, 1), f32),
        "invd_mats": np.stack([np.full((128, 128), 1.0 / D, f32),
                               np.full((128, 128), 1.0 / DH, f32)]),
        "bcast_rows": np.stack([
            np.full(128, 1.0 / D, f32),
            np.full(128, 1.0 / DH, f32),
            np.full(128, np.sqrt(float(D)), f32),
            np.full(128, np.sqrt(float(DH)), f32),
        ]),
    }
    # head layers: 0=sign, 1=aux, 2..7 = digit p
    lw = np.empty((NL, D, DH), f32)
    lb = np.empty((NL, DH), f32)
    lg = np.ones((NL, DH), f32)
    lbe = np.zeros((NL, DH), f32)
    w2 = np.zeros((NL, DH, 16), f32)
    b2 = np.zeros((NL, 16), f32)
    lw[0] = inputs["sign_w1"]; lb[0] = inputs["sign_b1"]
    lg[0] = inputs["sign_g"]; lbe[0] = inputs["sign_beta"]
    w2[0, :, :3] = inputs["sign_w2"]; b2[0, :3] = inputs["sign_b2"]
    lw[7] = inputs["aux_w1"]; lb[7] = inputs["aux_b1"]
    w2[7, :, :1] = inputs["aux_w2"]; b2[7, :1] = inputs["aux_b2"]
    pos_embed = np.asarray(inputs["pos_embed"], f32)
    dig_b1_eff = np.asarray(inputs["dig_b1"], f32) + np.einsum(
        "pq,pqh->ph", pos_embed, dig_w1[:, D:, :])
    for p in range(P):
        lw[1 + p] = dig_w1[p, :D]
        lb[1 + p] = dig_b1_eff[p]
        lg[1 + p] = np.asarray(inputs["dig_g"], f32)[p]
        lbe[1 + p] = np.asarray(inputs["dig_beta"], f32)[p]
        w2[1 + p, :, :10] = np.asarray(inputs["dig_w2"], f32)[p]
        b2[1 + p, :10] = np.asarray(inputs["dig_b2"], f32)[p]
    shared.update(lw_all=lw, lb_all=lb, lg_all=lg, lbeta_all=lbe,
                  w2_all=w2, b2_all=b2)

    w_op = ctx_w[D:D + DQ]
    op_embed = np.asarray(inputs["op_embed"], f32)
    ctx_b = np.asarray(inputs["ctx_b"], f32)
    eye = np.eye(NPT, dtype=f32)

    in_maps = []
    for c in range(N_CORES):
        m = dict(shared)
        m["hiddenT"] = np.ascontiguousarray(hidden[c].T)
        m["onehot"] = np.ascontiguousarray(eye[pt[c]].T)
        m["ctx_bias"] = ctx_b + op_embed[op_t[c]] @ w_op
        in_maps.append(m)
    return in_maps


_NC_CACHE = {}


def _get_nc():
    if "nc" not in _NC_CACHE:
        _NC_CACHE["nc"] = build_program()
    return _NC_CACHE["nc"]


def _postprocess(results):
    f32 = np.float32
    sign = np.empty((B, T, 3), f32)
    digit = np.empty((B, T, P, 10), f32)
    aux = np.empty((B, T, 1), f32)
    for c in range(N_CORES):
        r = results[c]
        sign[c] = r["out_sign"].T
        aux[c] = r["out_aux"].T
        digit[c] = r["out_dig"].transpose(2, 0, 1)
    return sign, digit, aux


def kernel(**inputs):
    nc = _get_nc()
    in_maps = _preprocess(inputs)
    res = run_bass_kernel_spmd(nc, in_maps, core_ids=list(range(N_CORES)))
    return _postprocess(res.results)


def _install_ntff_hook_shim():
    """The image's antenv lacks axon_hooks; recreate it (see
    trn_agent_boot.trn_boot._ntff_profile_via_ctypes)."""
    import sys, types, ctypes, contextlib

    if "antenv.axon_hooks" in sys.modules:
        return
    lib = ctypes.CDLL("/opt/axon/libaxon_pjrt.so")
    if not hasattr(lib, "axon_start_nrt_profile"):
        return
    lib.axon_start_nrt_profile.argtypes = [ctypes.POINTER(ctypes.c_int64),
                                           ctypes.c_size_t]
    lib.axon_start_nrt_profile.restype = ctypes.c_int64
    lib.axon_stop_nrt_profile.argtypes = [ctypes.c_char_p]
    lib.axon_stop_nrt_profile.restype = ctypes.c_int64

    @contextlib.contextmanager
    def _hook(output_dir, device_ids):
        import jax
        jax.devices()
        if device_ids:
            ids = (ctypes.c_int64 * len(device_ids))(*device_ids)
            rc = lib.axon_start_nrt_profile(ids, len(device_ids))
        else:
            rc = lib.axon_start_nrt_profile(None, 0)
        if rc != 0:
            raise RuntimeError(f"axon_start_nrt_profile rc={rc}")
        try:
            yield
        finally:
            n = lib.axon_stop_nrt_profile(str(output_dir).encode())
            print(f"ntff profile: {n} file(s) written to {output_dir}")

    mod = types.ModuleType("antenv.axon_hooks")
    mod.get_axon_ntff_profile_hook = lambda: _hook
    mod.set_axon_ntff_profile_hook = lambda h: None
    sys.modules["antenv.axon_hooks"] = mod


def kernel_profiled(**inputs):
    """Like kernel() but returns (outputs, exec_time_ns)."""
    _install_ntff_hook_shim()
    nc = _get_nc()
    in_maps = _preprocess(inputs)
    res = run_bass_kernel_spmd(nc, in_maps, core_ids=list(range(N_CORES)),
                               trace=True)
    return _postprocess(res.results), res.exec_time_ns


# revision 31
# speedup vs baseline: 1.0313x; 1.0313x over previous
"""Trainium2 Bass kernel for nn_DigitByDigitValueHead.

Model (per token):
  ctx_in = concat(hidden, op_emb, pm_emb)            # [1536]
  context = gelu(LN(ctx_in @ ctx_w + ctx_b))         # [1024]
  sign    = gelu(LN(context @ sign_w1 + b)) @ sign_w2 + b2     # [3]
  digit_p = gelu(LN(context @ dig_w1[p] (+pos term) + b)) @ dig_w2[p] + b2
  aux     = gelu(context @ aux_w1 + b) @ aux_w2 + b2            # [1]

Sharding: core c handles batch row b=c (2048 tokens). operation_type is
per-row, so its embedding contribution folds into a per-core bias vector
(host precompute). pt_embed folds into PM = pt_embed @ ctx_w[1280:1536]
([10,1024]) gathered on-device via a one-hot matmul accumulated into the
same PSUM group as hidden @ ctx_w[:1024].

On-chip layout is feature-major ([feat partitions, tokens free]); hidden is
transposed host-side. All matmuls run fp32r (1 cycle/row at N=512).
LayerNorm reduces over the feature (partition) axis: column sums of x and
x^2 via ones-matmuls into [1,N] PSUM rows; the [1,N] rows are repacked to
[128,N/128] via a DRAM round-trip so the Newton-rsqrt finalize runs
partition-parallel on DVE; mu and rsqrt(v+eps) are then broadcast across
partitions with scaled-ones [1,128] matmuls into PSUM, ACT-evicted, and
applied by DVE; gelu (+LN affine) runs on ACT (all ACT funcs used -
Identity/Copy-like evicts, Square, Gelu - live in one table set).
"""

import numpy as np
from contextlib import ExitStack

import concourse.bacc as bacc
import concourse.tile as tile
from concourse import mybir
from concourse.bass_utils import run_bass_kernel_spmd

F32 = mybir.dt.float32
F32R = mybir.dt.float32r
I32 = mybir.dt.int32
AF = mybir.ActivationFunctionType
ALU = mybir.AluOpType

B, T, D = 8, 2048, 1024
DQ, DH, P = 256, 512, 6
NOPS, NPT = 9, 10
N_CORES = 8
EPS = 1e-5
MAGIC = 0x5F3759DF
NT = 512                  # tokens per tile (fp32 matmul free-dim max)
NL = 8                    # head layers: 0=sign, 1..6=digit p, 7=aux
AUX_L = 7
USE_GELU = True           # CoreSim has no Gelu; tests may flip to Identity

# bcast_rows input: [4, 128] with rows (1/1024, 1/512, sqrt(1024), sqrt(512))
BC_INVD_CTX, BC_INVD_HEAD, BC_SQD_CTX, BC_SQD_HEAD = range(4)


def _ln_finalize(nc, sb, magic, mu_pk, q_pk, dim, r_out):
    """Packed mean MU and column sumsq Q [128,J] ->
    r_out = rsqrt(Q - D*MU^2 + eps*D) = rsqrt(D*(var+eps)); 3 Newton
    iterations from the int bit-trick seed. r_out is F32R (PE-consumed)."""
    J = mu_pk.shape[1]
    dd = float(dim)
    tmp = sb.tile([128, J], F32, tag="fin_tmp")
    w = sb.tile([128, J], F32, tag="fin_w")
    nc.vector.tensor_mul(tmp, mu_pk, mu_pk)
    nc.vector.scalar_tensor_tensor(w, tmp, -dd, q_pk,
                                   op0=ALU.mult, op1=ALU.add)
    nc.vector.tensor_scalar_add(w, w, EPS * dd)
    ti = sb.tile([128, J], I32, tag="fin_ti")
    nc.vector.tensor_scalar(ti, w.bitcast(I32), 1, None,
                            op0=ALU.logical_shift_right)
    nc.vector.scalar_tensor_tensor(ti, ti, -1, magic, op0=ALU.mult, op1=ALU.add)
    r = ti.bitcast(F32)
    for it in range(3):
        nc.vector.tensor_mul(tmp, r, r)
        nc.vector.scalar_tensor_tensor(tmp, w, -0.5, tmp,
                                       op0=ALU.mult, op1=ALU.mult)
        nc.vector.tensor_scalar_add(tmp, tmp, 1.5)
        nc.vector.tensor_mul(r_out if it == 2 else r, r, tmp)


def build_program(t_core=T, n_layers=NL, debug=False):
    """Build the per-core Bacc program. t_core tokens, feature-major I/O."""
    nc = bacc.Bacc("TRN2", target_bir_lowering=False, debug=debug,
                   num_devices=N_CORES)
    ntt = t_core // NT
    gelu_fn = AF.Gelu if USE_GELU else AF.Identity

    # ---- I/O ----
    hiddenT = nc.dram_tensor("hiddenT", [D, t_core], F32R, kind="ExternalInput").ap()
    onehot = nc.dram_tensor("onehot", [NPT, t_core], F32R, kind="ExternalInput").ap()
    ctx_bias = nc.dram_tensor("ctx_bias", [D], F32, kind="ExternalInput").ap()
    ctx_g = nc.dram_tensor("ctx_g", [D], F32, kind="ExternalInput").ap()
    ctx_beta = nc.dram_tensor("ctx_beta", [D], F32, kind="ExternalInput").ap()
    wh = nc.dram_tensor("wh", [D, D], F32R, kind="ExternalInput").ap()
    pm = nc.dram_tensor("pm", [NPT, D], F32R, kind="ExternalInput").ap()
    lw_all = nc.dram_tensor("lw_all", [n_layers, D, DH], F32R, kind="ExternalInput").ap()
    lb_all = nc.dram_tensor("lb_all", [n_layers, DH], F32, kind="ExternalInput").ap()
    lg_all = nc.dram_tensor("lg_all", [n_layers, DH], F32, kind="ExternalInput").ap()
    lbeta_all = nc.dram_tensor("lbeta_all", [n_layers, DH], F32, kind="ExternalInput").ap()
    # w2 columns per layer: sign 3, aux 1, digit 10 -> padded to 16 cols
    w2_all = nc.dram_tensor("w2_all", [n_layers, DH, 16], F32R, kind="ExternalInput").ap()
    b2_all = nc.dram_tensor("b2_all", [n_layers, 16], F32, kind="ExternalInput").ap()
    ones_col = nc.dram_tensor("ones_col", [128, 1], F32R, kind="ExternalInput").ap()
    invd_mats = nc.dram_tensor("invd_mats", [2, 128, 128], F32R, kind="ExternalInput").ap()
    bcast_rows = nc.dram_tensor("bcast_rows", [4, 128], F32R, kind="ExternalInput").ap()

    out_sign = nc.dram_tensor("out_sign", [3, t_core], F32, kind="ExternalOutput").ap()
    out_aux = nc.dram_tensor("out_aux", [1, t_core], F32, kind="ExternalOutput").ap()
    out_dig = nc.dram_tensor("out_dig", [P, 10, t_core], F32, kind="ExternalOutput").ap()
    outs = [out_sign] + [out_dig[p] for p in range(P)] + [out_aux]

    KC = D // 128   # 8 contraction chunks of context/hidden
    MC = D // 128   # 8 ctx output chunks
    HC = DH // 128  # 4 head chunks
    J = NT // 128   # 4

    with tile.TileContext(nc) as tc, ExitStack() as ctx:
        const = ctx.enter_context(tc.tile_pool(name="const", bufs=1))
        sbs = ctx.enter_context(tc.tile_pool(name="sbs", bufs=1))       # stat rows
        fin = ctx.enter_context(tc.tile_pool(name="fin", bufs=2))       # finalize
        dram = ctx.enter_context(tc.tile_pool(name="dram", bufs=4, space="DRAM"))
        ps_mm = ctx.enter_context(tc.tile_pool(name="ps_mm", bufs=3, space="PSUM"))
        ps_st = ctx.enter_context(tc.tile_pool(name="ps_st", bufs=1, space="PSUM"))
        ps_bcm = ctx.enter_context(tc.tile_pool(name="ps_bcm", bufs=2, space="PSUM"))
        ps_bcr = ctx.enter_context(tc.tile_pool(name="ps_bcr", bufs=1, space="PSUM"))
        ps_w2 = ctx.enter_context(tc.tile_pool(name="ps_w2", bufs=1, space="PSUM"))
        ctxp = ctx.enter_context(tc.tile_pool(name="ctxp", bufs=1))
        wtp = ctx.enter_context(tc.tile_pool(name="wtp", bufs=3))       # wh halves + lw stream
        htp = ctx.enter_context(tc.tile_pool(name="htp", bufs=2))       # hiddenT chunks
        xhp = ctx.enter_context(tc.tile_pool(name="xhp", bufs=2))       # pre-LN x / h tiles
        x2p = ctx.enter_context(tc.tile_pool(name="x2p", bufs=2))
        ohp = ctx.enter_context(tc.tile_pool(name="ohp", bufs=2))
        outp = ctx.enter_context(tc.tile_pool(name="outp", bufs=3))

        hid_r = hiddenT.rearrange("(kc kp) t -> kp kc t", kp=128)

        def load_hT(tt):
            tsl = slice(tt * NT, (tt + 1) * NT)
            tiles = []
            for k in range(KC):
                t = htp.tile([128, NT], F32R, tag=f"hT{k}")
                nc.sync.dma_start(out=t, in_=hid_r[:, k, tsl])
                tiles.append(t)
            return tiles

        # critical-path loads first: hT0/wh chunk pairs, interleaved so they
        # spread across DMA queues
        wh_r = wh.rearrange("(kc kp) m -> kp kc m", kp=128)
        wh_ab = []
        for i in range(2):
            wh_half = wtp.tile([128, KC // 2, D], F32R, tag="wt")
            wh_ab.append(wh_half)
        hT0 = []
        for k in range(KC):
            t = htp.tile([128, NT], F32R, tag=f"hT{k}")
            nc.sync.dma_start(out=t, in_=hid_r[:, k, 0:NT])
            hT0.append(t)
            nc.sync.dma_start(out=wh_ab[k // 4][:, k % 4, :], in_=wh_r[:, k, :])

        magic = const.tile([128, J], I32)
        nc.vector.memset(magic, MAGIC)
        ones_sb = const.tile([128, 1], F32R)
        nc.sync.dma_start(out=ones_sb, in_=ones_col)
        bcr_sb = []
        for i in range(4):
            t = const.tile([1, 128], F32R, tag=f"bcr{i}")
            nc.sync.dma_start(out=t, in_=bcast_rows[i:i + 1, :])
            bcr_sb.append(t)
        cb_sb = const.tile([128, MC], F32)
        nc.sync.dma_start(out=cb_sb, in_=ctx_bias.rearrange("(c p) -> p c", p=128))
        invd_sb = []
        for i in range(2):
            t = const.tile([128, 128], F32R, tag=f"invd{i}")
            nc.sync.dma_start(out=t, in_=invd_mats[i])
            invd_sb.append(t)
        pm_sb = const.tile([NPT, D], F32R)
        nc.sync.dma_start(out=pm_sb, in_=pm)

        def load_oh(tt):
            tsl = slice(tt * NT, (tt + 1) * NT)
            t = ohp.tile([NPT, NT], F32R, tag="oh")
            nc.sync.dma_start(out=t, in_=onehot[:, tsl])
            return t

        oh0 = load_oh(0)

        # remaining (non-critical) constants
        cg_sb = const.tile([128, MC], F32)
        nc.sync.dma_start(out=cg_sb, in_=ctx_g.rearrange("(c p) -> p c", p=128))
        cbe_sb = const.tile([128, MC], F32)
        nc.sync.dma_start(out=cbe_sb, in_=ctx_beta.rearrange("(c p) -> p c", p=128))
        lb_sb = const.tile([128, n_layers, HC], F32)
        nc.sync.dma_start(out=lb_sb, in_=lb_all.rearrange("l (c p) -> p l c", p=128))
        lg_sb = const.tile([128, n_layers, HC], F32)
        nc.sync.dma_start(out=lg_sb, in_=lg_all.rearrange("l (c p) -> p l c", p=128))
        lbe_sb = const.tile([128, n_layers, HC], F32)
        nc.sync.dma_start(out=lbe_sb, in_=lbeta_all.rearrange("l (c p) -> p l c", p=128))
        w2_sb = const.tile([128, n_layers, HC, 16], F32R)
        nc.sync.dma_start(out=w2_sb, in_=w2_all.rearrange("l (c p) o -> p l c o", p=128))
        b2_sb = const.tile([16, n_layers], F32)
        nc.sync.dma_start(out=b2_sb, in_=b2_all.rearrange("l o -> o l"))

        # context, resident (written as F32R by ACT gelu); one tile per
        # token-tile so head-layer reads only depend on that tile's writers
        context_t = []
        for i in range(ntt):
            ct = ctxp.tile([128, KC, NT], F32R, tag=f"ctx{i}")
            context_t.append(ct)

        def ln_chain(mu_row, q_row, dim, bc_sqd):
            """mu/sumsq rows [1,NT] -> rps [128,NT] PSUM broadcast of
            rsqrt(var+eps)."""
            scr_s = dram.tile([NT], F32, tag="scr_s")
            nc.sync.dma_start(out=scr_s.unsqueeze(0), in_=mu_row)
            scr_q = dram.tile([NT], F32, tag="scr_q")
            nc.sync.dma_start(out=scr_q.unsqueeze(0), in_=q_row)
            s_pk = fin.tile([128, J], F32, tag="s_pk")
            nc.sync.dma_start(out=s_pk, in_=scr_s.rearrange("(p j) -> p j", p=128))
            q_pk = fin.tile([128, J], F32, tag="q_pk")
            nc.sync.dma_start(out=q_pk, in_=scr_q.rearrange("(p j) -> p j", p=128))
            r_pk = fin.tile([128, J], F32R, tag="r_pk")
            _ln_finalize(nc, fin, magic, s_pk, q_pk, dim, r_pk)
            scr_r = dram.tile([NT], F32R, tag="scr_r")
            nc.sync.dma_start(out=scr_r.rearrange("(p j) -> p j", p=128), in_=r_pk)
            r_row = sbs.tile([1, NT], F32R, tag="r_row")
            nc.sync.dma_start(out=r_row, in_=scr_r.unsqueeze(0))
            rps = ps_bcr.tile([128, NT], F32, tag="rps")
            nc.tensor.matmul(rps, bcr_sb[bc_sqd], r_row,
                             start=True, stop=True)
            return rps

        # ==================== Phase 1: ctx layer ====================
        hts = {0: hT0}
        ohs = {0: oh0}
        for tt in range(ntt):
            tsl = slice(tt * NT, (tt + 1) * NT)
            hT = hts.pop(tt)
            oh_t = ohs.pop(tt)
            if tt + 1 < ntt:
                hts[tt + 1] = load_hT(tt + 1)
                ohs[tt + 1] = load_oh(tt + 1)

            mups = ps_bcm.tile([128, NT], F32, tag="mups")
            qps = ps_st.tile([1, NT], F32, tag="qps")
            xc = []
            x2c = []
            for m in range(MC):
                psum = ps_mm.tile([128, NT], F32, tag="mm")
                msl = slice(m * 128, (m + 1) * 128)
                for k in range(KC):
                    nc.tensor.matmul(psum, wh_ab[k // 4][:, k % 4, msl], hT[k],
                                     start=(k == 0), stop=False)
                nc.tensor.matmul(psum, pm_sb[:, msl], oh_t,
                                 start=False, stop=True)
                x = xhp.tile([128, NT], F32R, tag=f"xh{m}")
                nc.scalar.activation(x, psum, AF.Identity,
                                     bias=cb_sb[:, m:m + 1], scale=1.0)
                x2 = x2p.tile([128, NT], F32R, tag="x2")
                nc.scalar.activation(x2, psum, AF.Square,
                                     bias=cb_sb[:, m:m + 1], scale=1.0)
                nc.tensor.matmul(qps, ones_sb, x2,
                                 start=(m == 0), stop=(m == MC - 1))
                xc.append(x)

            for m in range(MC):
                nc.tensor.matmul(mups, invd_sb[0], xc[m],
                                 start=(m == 0), stop=(m == MC - 1))
            mu_row = sbs.tile([1, NT], F32, tag="mu_row")
            q_row = sbs.tile([1, NT], F32, tag="q_row")
            nc.vector.tensor_copy(mu_row, mups[0:1, :])
            nc.vector.tensor_copy(q_row, qps)
            r_sb = ln_chain(mu_row, q_row, D, BC_SQD_CTX)
            mu_sb = mups
            for m in range(MC):
                x = xc[m]
                nc.vector.tensor_sub(x, x.bitcast(F32), mu_sb)
                nc.vector.tensor_mul(x, x.bitcast(F32), r_sb)
                nc.scalar.activation(context_t[tt][:, m, :], x.bitcast(F32), gelu_fn,
                                     bias=cbe_sb[:, m:m + 1],
                                     scale=cg_sb[:, m:m + 1])

        # ==================== Phase 2: head layers ====================
        for L in range(n_layers):
            lw = wtp.tile([128, KC, DH], F32R, tag="wt")
            nc.sync.dma_start(
                out=lw, in_=lw_all[L].rearrange("(kc kp) m -> kp kc m", kp=128))
            n_out = 3 if L == 0 else (1 if L == AUX_L else 10)
            for tt in range(ntt):
                tsl = slice(tt * NT, (tt + 1) * NT)
                hs = []
                x2s = []
                if L != AUX_L:
                    mups = ps_bcm.tile([128, NT], F32, tag="mups")
                    qps = ps_st.tile([1, NT], F32, tag="qps")
                for m in range(HC):
                    psum = ps_mm.tile([128, NT], F32, tag="mm")
                    msl = slice(m * 128, (m + 1) * 128)
                    for k in range(KC):
                        nc.tensor.matmul(psum, lw[:, k, msl],
                                         context_t[tt][:, k, :],
                                         start=(k == 0), stop=(k == KC - 1))
                    h = xhp.tile([128, NT], F32R, tag=f"xh{m + 4 * (tt % 2)}")
                    if L == AUX_L:
                        nc.scalar.activation(h, psum, gelu_fn,
                                             bias=lb_sb[:, L, m:m + 1], scale=1.0)
                    else:
                        nc.scalar.activation(h, psum, AF.Identity,
                                             bias=lb_sb[:, L, m:m + 1], scale=1.0)
                        x2 = x2p.tile([128, NT], F32R, tag="x2")
                        nc.scalar.activation(x2, psum, AF.Square,
                                             bias=lb_sb[:, L, m:m + 1], scale=1.0)
                        nc.tensor.matmul(qps, ones_sb, x2,
                                         start=(m == 0), stop=(m == HC - 1))
                    hs.append(h)

                if L != AUX_L:
                    for m in range(HC):
                        nc.tensor.matmul(mups, invd_sb[1], hs[m],
                                         start=(m == 0), stop=(m == HC - 1))
                    mu_row = sbs.tile([1, NT], F32, tag="mu_row")
                    q_row = sbs.tile([1, NT], F32, tag="q_row")
                    nc.vector.tensor_copy(mu_row, mups[0:1, :])
                    nc.vector.tensor_copy(q_row, qps)
                    r_sb = ln_chain(mu_row, q_row, DH, BC_SQD_HEAD)
                    mu_sb = mups
                    for m in range(HC):
                        h = hs[m]
                        nc.vector.tensor_sub(h, h.bitcast(F32), mu_sb)
                        nc.vector.tensor_mul(h, h.bitcast(F32), r_sb)
                        nc.scalar.activation(h, h.bitcast(F32), gelu_fn,
                                             bias=lbe_sb[:, L, m:m + 1],
                                             scale=lg_sb[:, L, m:m + 1])

                psum2 = ps_w2.tile([n_out, NT], F32, tag="w2")
                for k in range(HC):
                    nc.tensor.matmul(psum2, w2_sb[:, L, k, :n_out], hs[k],
                                     start=(k == 0), stop=(k == HC - 1))
                ostage = outp.tile([n_out, NT], F32, tag="ost")
                nc.scalar.activation(ostage, psum2, AF.Identity,
                                     bias=b2_sb[:n_out, L:L + 1], scale=1.0)
                nc.sync.dma_start(out=outs[L][:, tsl], in_=ostage)

    nc.compile()
    return nc


# ----------------------------------------------------------------------------
# Host side
# ----------------------------------------------------------------------------

def _preprocess(inputs):
    f32 = np.float32
    hidden = np.asarray(inputs["hidden"], f32)
    op_t = np.asarray(inputs["operation_type"])
    pt = np.asarray(inputs["param_type"])
    ctx_w = np.asarray(inputs["ctx_w"], f32)
    dig_w1 = np.asarray(inputs["dig_w1"], f32)

    shared = {
        "wh": np.ascontiguousarray(ctx_w[:D]),
        "pm": np.ascontiguousarray(np.asarray(inputs["pt_embed"], f32) @ ctx_w[D + DQ:]),
        "ctx_g": np.asarray(inputs["ctx_g"], f32),
        "ctx_beta": np.asarray(inputs["ctx_beta"], f32),
        "ones_col": np.ones((128, 1), f32),
        "invd_mats": np.stack([np.full((128, 128), 1.0 / D, f32),
                               np.full((128, 128), 1.0 / DH, f32)]),
        "bcast_rows": np.stack([
            np.full(128, 1.0 / D, f32),
            np.full(128, 1.0 / DH, f32),
            np.full(128, np.sqrt(float(D)), f32),
            np.full(128, np.sqrt(float(DH)), f32),
        ]),
    }
    # head layers: 0=sign, 1=aux, 2..7 = digit p
    lw = np.empty((NL, D, DH), f32)
    lb = np.empty((NL, DH), f32)
    lg = np.ones((NL, DH), f32)
    lbe = np.zeros((NL, DH), f32)
    w2 = np.zeros((NL, DH, 16), f32)
    b2 = np.zeros((NL, 16), f32)
    lw[0] = inputs["sign_w1"]; lb[0] = inputs["sign_b1"]
    lg[0] = inputs["sign_g"]; lbe[0] = inputs["sign_beta"]
    w2[0, :, :3] = inputs["sign_w2"]; b2[0, :3] = inputs["sign_b2"]
    lw[7] = inputs["aux_w1"]; lb[7] = inputs["aux_b1"]
    w2[7, :, :1] = inputs["aux_w2"]; b2[7, :1] = inputs["aux_b2"]
    pos_embed = np.asarray(inputs["pos_embed"], f32)
    dig_b1_eff = np.asarray(inputs["dig_b1"], f32) + np.einsum(
        "pq,pqh->ph", pos_embed, dig_w1[:, D:, :])
    for p in range(P):
        lw[1 + p] = dig_w1[p, :D]
        lb[1 + p] = dig_b1_eff[p]
        lg[1 + p] = np.asarray(inputs["dig_g"], f32)[p]
        lbe[1 + p] = np.asarray(inputs["dig_beta"], f32)[p]
        w2[1 + p, :, :10] = np.asarray(inputs["dig_w2"], f32)[p]
        b2[1 + p, :10] = np.asarray(inputs["dig_b2"], f32)[p]
    shared.update(lw_all=lw, lb_all=lb, lg_all=lg, lbeta_all=lbe,
                  w2_all=w2, b2_all=b2)

    w_op = ctx_w[D:D + DQ]
    op_embed = np.asarray(inputs["op_embed"], f32)
    ctx_b = np.asarray(inputs["ctx_b"], f32)
    eye = np.eye(NPT, dtype=f32)

    in_maps = []
    for c in range(N_CORES):
        m = dict(shared)
        m["hiddenT"] = np.ascontiguousarray(hidden[c].T)
        m["onehot"] = np.ascontiguousarray(eye[pt[c]].T)
        m["ctx_bias"] = ctx_b + op_embed[op_t[c]] @ w_op
        in_maps.append(m)
    return in_maps


_NC_CACHE = {}


def _get_nc():
    if "nc" not in _NC_CACHE:
        _NC_CACHE["nc"] = build_program()
    return _NC_CACHE["nc"]


def _postprocess(results):
    f32 = np.float32
    sign = np.empty((B, T, 3), f32)
    digit = np.empty((B, T, P, 10), f32)
    aux = np.empty((B, T, 1), f32)
    for c in range(N_CORES):
        r = results[c]
        sign[c] = r["out_sign"].T
        aux[c] = r["out_aux"].T
        digit[c] = r["out_dig"].transpose(2, 0, 1)
    return sign, digit, aux


def kernel(**inputs):
    nc = _get_nc()
    in_maps = _preprocess(inputs)
    res = run_bass_kernel_spmd(nc, in_maps, core_ids=list(range(N_CORES)))
    return _postprocess(res.results)


def _install_ntff_hook_shim():
    """The image's antenv lacks axon_hooks; recreate it (see
    trn_agent_boot.trn_boot._ntff_profile_via_ctypes)."""
    import sys, types, ctypes, contextlib

    if "antenv.axon_hooks" in sys.modules:
        return
    lib = ctypes.CDLL("/opt/axon/libaxon_pjrt.so")
    if not hasattr(lib, "axon_start_nrt_profile"):
        return
    lib.axon_start_nrt_profile.argtypes = [ctypes.POINTER(ctypes.c_int64),
                                           ctypes.c_size_t]
    lib.axon_start_nrt_profile.restype = ctypes.c_int64
    lib.axon_stop_nrt_profile.argtypes = [ctypes.c_char_p]
    lib.axon_stop_nrt_profile.restype = ctypes.c_int64

    @contextlib.contextmanager
    def _hook(output_dir, device_ids):
        import jax
        jax.devices()
        if device_ids:
            ids = (ctypes.c_int64 * len(device_ids))(*device_ids)
            rc = lib.axon_start_nrt_profile(ids, len(device_ids))
        else:
            rc = lib.axon_start_nrt_profile(None, 0)
        if rc != 0:
            raise RuntimeError(f"axon_start_nrt_profile rc={rc}")
        try:
            yield
        finally:
            n = lib.axon_stop_nrt_profile(str(output_dir).encode())
            print(f"ntff profile: {n} file(s) written to {output_dir}")

    mod = types.ModuleType("antenv.axon_hooks")
    mod.get_axon_ntff_profile_hook = lambda: _hook
    mod.set_axon_ntff_profile_hook = lambda h: None
    sys.modules["antenv.axon_hooks"] = mod


def kernel_profiled(**inputs):
    """Like kernel() but returns (outputs, exec_time_ns)."""
    _install_ntff_hook_shim()
    nc = _get_nc()
    in_maps = _preprocess(inputs)
    res = run_bass_kernel_spmd(nc, in_maps, core_ids=list(range(N_CORES)),
                               trace=True)
    return _postprocess(res.results), res.exec_time_ns
